# revision 11
# baseline (speedup 1.0000x reference)
"""GCN-over-bipartite-batch kernel for 8 Trainium2 NeuronCores.

The reference graph (user_i <-> pos_i edges + self-loops on all 3B nodes)
has deterministic degrees (user/pos: 2, neg: 1), so GCNConv collapses to
dense per-row math with no scatter:

    u = pos = relu(0.5*(user_e + pos_e) @ W1 + b1) @ W2 + b2
    neg     = relu(neg_e @ W1 + b1) @ W2 + b2
    pos_score = sum(u*u, -1); neg_score = sum(u*neg, -1)
    outputs: (pos_score [B,1], neg_score [B,1], cat([pos,neg]) [B,128], u [B,64])

Distribution: embedding tables replicated to all 8 cores, batch split
8 x 8192 data-parallel (no collectives). Per core the kernel is a
gather (indirect DMA) -> transpose -> 2 tiny matmuls -> transpose -> store
pipeline, memory-bound on the 3*8192 random 256B row gathers.
"""

import numpy as np

import concourse.bacc as bacc
import concourse.bass as bass
import concourse.mybir as mybir
import concourse.tile as tile
from concourse.bass_utils import run_bass_kernel_spmd
from concourse.masks import make_identity

N_CORES = 8
D = 64            # hidden channels
P = 128           # SBUF partitions
B = 65536         # full batch
NU = 1_000_000    # user table rows
NI = 1_000_000    # item table rows
BC = B // N_CORES # batch per core (8192)

F32 = mybir.dt.float32
I32 = mybir.dt.int32

RELU = mybir.ActivationFunctionType.Relu
IDENT = mybir.ActivationFunctionType.Identity


def build_nc(nu=NU, ni=NI, bc=BC, gather_chunk=2048, compute_chunk=512):
    """Build the per-core Bass program (SPMD: all 8 cores run this)."""
    n_tiles = bc // P                       # 128-row batch tiles per core
    gt = gather_chunk // P                  # tiles per gather chunk
    ct = compute_chunk // P                 # tiles per compute chunk
    n_gc = bc // gather_chunk               # gather chunks
    n_cc = gather_chunk // compute_chunk    # compute chunks per gather chunk
    assert bc % gather_chunk == 0 and gather_chunk % compute_chunk == 0

    nc = bacc.Bacc(None, target_bir_lowering=False)

    uemb = nc.dram_tensor("uemb", [nu, D], F32, kind="ExternalInput")
    iemb = nc.dram_tensor("iemb", [ni, D], F32, kind="ExternalInput")
    # index layout [P, n_tiles]: element [p, t] = batch position t*128+p
    uidx = nc.dram_tensor("uidx", [P, n_tiles], I32, kind="ExternalInput")
    pidx = nc.dram_tensor("pidx", [P, n_tiles], I32, kind="ExternalInput")
    nidx = nc.dram_tensor("nidx", [P, n_tiles], I32, kind="ExternalInput")
    w11 = nc.dram_tensor("w11", [P, D], F32, kind="ExternalInput")   # [0.5*W1; W1]
    w22 = nc.dram_tensor("w22", [P, D], F32, kind="ExternalInput")   # [W2; W2]
    b1d = nc.dram_tensor("b1d", [P, 1], F32, kind="ExternalInput")   # [b1;b1]
    b2d = nc.dram_tensor("b2d", [P, 1], F32, kind="ExternalInput")   # [b2;b2]

    pos_s = nc.dram_tensor("pos_s", [bc], F32, kind="ExternalOutput")
    neg_s = nc.dram_tensor("neg_s", [bc], F32, kind="ExternalOutput")
    cat = nc.dram_tensor("cat", [bc, 2 * D], F32, kind="ExternalOutput")
    u_out = nc.dram_tensor("u_out", [bc, D], F32, kind="ExternalOutput")

    with tile.TileContext(nc) as tc:
        with (
            tc.tile_pool(name="const", bufs=1) as cpool,
            tc.tile_pool(name="gather", bufs=2) as gpool,
            tc.tile_pool(name="work", bufs=3) as wpool,
            tc.tile_pool(name="psA", bufs=2, space="PSUM") as psA,
            tc.tile_pool(name="psB", bufs=2, space="PSUM") as psB,
        ):
            ident = cpool.tile([P, P], F32)
            make_identity(nc, ident[:])
            w11_sb = cpool.tile([P, D], F32)
            nc.sync.dma_start(w11_sb[:], w11[:])
            w22_sb = cpool.tile([P, D], F32)
            nc.sync.dma_start(w22_sb[:], w22[:])
            b1_sb = cpool.tile([P, 1], F32)
            nc.sync.dma_start(b1_sb[:], b1d[:])
            b2_sb = cpool.tile([P, 1], F32)
            nc.sync.dma_start(b2_sb[:], b2d[:])
            uidx_sb = cpool.tile([P, n_tiles], I32)
            nc.sync.dma_start(uidx_sb[:], uidx[:])
            pidx_sb = cpool.tile([P, n_tiles], I32)
            nc.sync.dma_start(pidx_sb[:], pidx[:])
            nidx_sb = cpool.tile([P, n_tiles], I32)
            nc.sync.dma_start(nidx_sb[:], nidx[:])

            for g in range(n_gc):
                t0 = g * gt
                gu = gpool.tile([P, gt * D], F32, tag="gu")
                gp = gpool.tile([P, gt * D], F32, tag="gp")
                # packed: per 128-col block t -> [a_t | gn_t]
                packed = gpool.tile([P, gt * 2 * D], F32, tag="packed")
                packed3 = packed[:].rearrange("p (t c) -> p t c", c=2 * D)
                # HW indirect DMA consumes exactly one index per partition,
                # so gathers go 128 rows (one batch tile) at a time.
                for t in range(gt):
                    nc.gpsimd.indirect_dma_start(
                        out=gu[:, t * D:(t + 1) * D], out_offset=None,
                        in_=uemb[:],
                        in_offset=bass.IndirectOffsetOnAxis(
                            ap=uidx_sb[:, t0 + t:t0 + t + 1], axis=0),
                    )
                    nc.gpsimd.indirect_dma_start(
                        out=gp[:, t * D:(t + 1) * D], out_offset=None,
                        in_=iemb[:],
                        in_offset=bass.IndirectOffsetOnAxis(
                            ap=pidx_sb[:, t0 + t:t0 + t + 1], axis=0),
                    )
                    nc.gpsimd.indirect_dma_start(
                        out=packed3[:, t, D:2 * D], out_offset=None,
                        in_=iemb[:],
                        in_offset=bass.IndirectOffsetOnAxis(
                            ap=nidx_sb[:, t0 + t:t0 + t + 1], axis=0),
                    )
                nc.vector.tensor_add(
                    out=packed3[:, :, 0:D],
                    in0=gu[:].rearrange("p (t c) -> p t c", c=D),
                    in1=gp[:].rearrange("p (t c) -> p t c", c=D),
                )

                for s in range(n_cc):
                    cc = compute_chunk
                    b0 = (g * n_cc + s) * cc  # batch offset within core
                    psumT = psA.tile([P, cc], F32, tag="psumT")
                    for t in range(ct):
                        blk = (s * ct + t) * P
                        nc.tensor.transpose(
                            out=psumT[:, t * P:(t + 1) * P],
                            in_=packed[:, blk:blk + P],
                            identity=ident[:],
                        )
                    # sbufT: rows 0:64 = a^T, rows 64:128 = gn^T (batch on free)
                    sbufT = wpool.tile([P, cc], F32, tag="sbufT")
                    nc.vector.tensor_copy(sbufT[:], psumT[:])

                    ph1 = psB.tile([P, cc], F32, tag="ph1")
                    nc.tensor.matmul(ph1[0:D, :], lhsT=w11_sb[0:D, :],
                                     rhs=sbufT[0:D, :], start=True, stop=True)
                    nc.tensor.matmul(ph1[D:P, :], lhsT=w11_sb[D:P, :],
                                     rhs=sbufT[D:P, :], start=True, stop=True)
                    h1 = wpool.tile([P, cc], F32, tag="h1")
                    nc.scalar.activation(h1[:], ph1[:], RELU, bias=b1_sb[:])

                    p2 = psB.tile([P, cc], F32, tag="p2")
                    nc.tensor.matmul(p2[0:D, :], lhsT=w22_sb[0:D, :],
                                     rhs=h1[0:D, :], start=True, stop=True)
                    nc.tensor.matmul(p2[D:P, :], lhsT=w22_sb[D:P, :],
                                     rhs=h1[D:P, :], start=True, stop=True)
                    # uN: rows 0:64 = u^T, rows 64:128 = neg^T
                    uN = wpool.tile([P, cc], F32, tag="uN")
                    nc.scalar.activation(uN[:], p2[:], IDENT, bias=b2_sb[:])

                    # transpose back: cat_sb rows = batch, cols = [u | neg]
                    pcat = psA.tile([P, cc], F32, tag="pcat")
                    for t in range(ct):
                        nc.tensor.transpose(
                            out=pcat[:, t * P:(t + 1) * P],
                            in_=uN[:, t * P:(t + 1) * P],
                            identity=ident[:],
                        )
                    cat_sb = wpool.tile([P, cc], F32, tag="cat_sb")
                    nc.vector.tensor_copy(cat_sb[:], pcat[:])

                    # scores in row layout: per 128-col block, cols 0:64 = u,
                    # 64:128 = neg for 128 batch rows
                    cat3 = cat_sb[:].rearrange("p (t c) -> p t c", c=P)
                    sq = wpool.tile([P, ct * D], F32, tag="sq")
                    pr = wpool.tile([P, ct * D], F32, tag="pr")
                    sq3 = sq[:].rearrange("p (t c) -> p t c", c=D)
                    pr3 = pr[:].rearrange("p (t c) -> p t c", c=D)
                    nc.vector.tensor_mul(sq3, cat3[:, :, 0:D], cat3[:, :, 0:D])
                    nc.vector.tensor_mul(pr3, cat3[:, :, 0:D], cat3[:, :, D:P])
                    scr = wpool.tile([P, 2 * ct], F32, tag="scr")
                    nc.vector.reduce_sum(scr[:, 0:ct], sq3,
                                         axis=mybir.AxisListType.X)
                    nc.vector.reduce_sum(scr[:, ct:2 * ct], pr3,
                                         axis=mybir.AxisListType.X)

                    # stores
                    nc.sync.dma_start(
                        cat[b0:b0 + cc, :].rearrange("(t p) c -> p t c", p=P),
                        cat_sb[:].rearrange("p (t c) -> p t c", c=P),
                    )
                    nc.sync.dma_start(
                        u_out[b0:b0 + cc, :].rearrange("(t p) c -> p t c", p=P),
                        cat3[:, :, 0:D],
                    )
                    nc.sync.dma_start(
                        pos_s[b0:b0 + cc].rearrange("(t p) -> p t", p=P),
                        scr[:, 0:ct],
                    )
                    nc.sync.dma_start(
                        neg_s[b0:b0 + cc].rearrange("(t p) -> p t", p=P),
                        scr[:, ct:2 * ct],
                    )
    nc.compile()
    return nc


_NC_CACHE = {}


def _get_nc(key, **kwargs):
    if key not in _NC_CACHE:
        _NC_CACHE[key] = build_nc(**kwargs)
    return _NC_CACHE[key]


def _prep_idx(idx, bc):
    # [bc] -> [128, bc//128] with [p, t] = idx[t*128 + p]
    return np.ascontiguousarray(
        np.asarray(idx, dtype=np.int64).astype(np.int32).reshape(bc // P, P).T
    )


def run(user, pos_item, neg_item, user_emb_w, item_emb_w, W1, b1, W2, b2,
        **spmd_kwargs):
    user_emb_w = np.ascontiguousarray(np.asarray(user_emb_w, dtype=np.float32))
    item_emb_w = np.ascontiguousarray(np.asarray(item_emb_w, dtype=np.float32))
    W1 = np.ascontiguousarray(np.asarray(W1, dtype=np.float32))
    W2 = np.ascontiguousarray(np.asarray(W2, dtype=np.float32))
    b1 = np.asarray(b1, dtype=np.float32)
    b2 = np.asarray(b2, dtype=np.float32)
    w11 = np.ascontiguousarray(np.concatenate([0.5 * W1, W1], axis=0))
    w22 = np.ascontiguousarray(np.concatenate([W2, W2], axis=0))
    b1d = np.ascontiguousarray(np.concatenate([b1, b1]).reshape(P, 1))
    b2d = np.ascontiguousarray(np.concatenate([b2, b2]).reshape(P, 1))

    nc = _get_nc("full")

    in_maps = []
    for c in range(N_CORES):
        sl = slice(c * BC, (c + 1) * BC)
        in_maps.append(dict(
            uemb=user_emb_w, iemb=item_emb_w,
            uidx=_prep_idx(np.asarray(user)[sl], BC),
            pidx=_prep_idx(np.asarray(pos_item)[sl], BC),
            nidx=_prep_idx(np.asarray(neg_item)[sl], BC),
            w11=w11, w22=w22, b1d=b1d, b2d=b2d,
        ))

    res = run_bass_kernel_spmd(nc, in_maps, list(range(N_CORES)), **spmd_kwargs)
    outs = res.results
    pos_score = np.concatenate([outs[c]["pos_s"] for c in range(N_CORES)])
    neg_score = np.concatenate([outs[c]["neg_s"] for c in range(N_CORES)])
    cat = np.concatenate([outs[c]["cat"] for c in range(N_CORES)], axis=0)
    u = np.concatenate([outs[c]["u_out"] for c in range(N_CORES)], axis=0)
    return (pos_score.reshape(B, 1), neg_score.reshape(B, 1), cat, u), res


def kernel(**inputs):
    outputs, _ = run(**inputs)
    return outputs
